# revision 2
# baseline (speedup 1.0000x reference)
"""Trainium2 Bass kernel for nn_CategoryMultiplier.

out[b, s, :] = inputs[b, s, :] * (emb_table[categories[b, s]] if
               categories[b, s] != 0 else 1.0)

Sharding: pure data parallel over batch. 8 cores x 16 batches each.

All device I/O is bf16 (harness gate is rel_err < 2e-2; bf16 rounding of
x, table and y is ~0.4% worst case): per core x flat [8192, 512] bf16,
table [1000, 512] bf16 (row 0 pre-set to ones on host so padding
positions multiply by 1.0), y [8192, 512] bf16 upcast to f32 on host.
This halves HBM/DMA traffic vs f32 (24MB/core vs 48MB/core).

Device layout: positions are partition-major (partition p holds positions
p*64 .. p*64+63) so the input/output DMAs use contiguous descriptors per
partition. Embedding rows are fetched with one InstDMAGatherAnt per chunk;
its fixed dst layout dst[i%128, i//128] is reconciled with the
partition-major layout by permuting the index array on the host (pure
layout prep).
"""

import numpy as np
import ml_dtypes

import concourse.bass as bass
import concourse.bacc as bacc
import concourse.mybir as mybir
import concourse.tile as tile
from concourse.bass_utils import run_bass_kernel_spmd

BF16 = ml_dtypes.bfloat16

# Problem shape (hardcoded per harness contract).
B, S, D = 128, 512, 512
VOCAB = 1000
N_CORES = 8
B_LOC = B // N_CORES            # 16 batches per core
N = B_LOC * S                   # 8192 positions per core
P = 128                         # SBUF partitions
C = N // P                      # 64 positions per partition
T_CH = 8                        # max positions-per-partition per chunk

MBF16 = mybir.dt.bfloat16
I16 = mybir.dt.int16

# Taper: small chunks at head (prime the pipeline) and tail (short drain).
CHUNKS = [4, 4] + [8] * 6 + [4, 4]
assert sum(CHUNKS) == C


def _build_nc():
    nc = bacc.Bacc("TRN2", target_bir_lowering=False, debug=False)

    x = nc.dram_tensor("x", [N, D], MBF16, kind="ExternalInput")
    cats16 = nc.dram_tensor("cats16", [P, N // 16], I16, kind="ExternalInput")
    table = nc.dram_tensor("table", [VOCAB, D], MBF16, kind="ExternalInput")
    y = nc.dram_tensor("y", [N, D], MBF16, kind="ExternalOutput")

    xr = x[:].rearrange("(p c) d -> p (c d)", p=P)     # [128, C*D]
    yr = y[:].rearrange("(p c) d -> p (c d)", p=P)

    # Issue the GPSIMD ucode library load BEFORE the TileContext so the
    # ~14us IRAM load overlaps Tile's own prologue barrier instead of
    # running after it.
    from concourse.library_config import mlp
    nc.gpsimd.load_library(mlp)

    with tile.TileContext(nc) as tc:
        with (
            tc.tile_pool(name="const", bufs=1) as const_pool,
            tc.tile_pool(name="io", bufs=5) as io_pool,
            tc.tile_pool(name="gat", bufs=6) as gat_pool,
        ):
            cats_t = const_pool.tile([P, N // 16], I16)
            nc.scalar.dma_start(out=cats_t[:], in_=cats16[:])

            pos = 0
            for ci, tch in enumerate(CHUNKS):
                lo, hi = pos * D, (pos + tch) * D
                n_idx = tch * P
                g_t = gat_pool.tile([P, T_CH * D], MBF16, tag="g")
                nc.gpsimd.dma_gather(
                    out_ap=g_t[:, :tch * D].rearrange("p (t d) -> p t d", t=tch),
                    in_ap=table[:],
                    idxs_ap=cats_t[:, pos * 8:(pos + tch) * 8],
                    num_idxs=n_idx,
                    num_idxs_reg=n_idx,
                    elem_size=D,
                )

                x_t = io_pool.tile([P, T_CH * D], MBF16, tag="x")
                nc.sync.dma_start(out=x_t[:, :tch * D], in_=xr[:, lo:hi])

                nc.vector.tensor_mul(out=g_t[:, :tch * D], in0=g_t[:, :tch * D],
                                     in1=x_t[:, :tch * D])
                nc.scalar.dma_start(out=yr[:, lo:hi], in_=g_t[:, :tch * D])
                pos += tch

    nc.compile()
    return nc


_NC = None


def _get_nc():
    global _NC
    if _NC is None:
        _NC = _build_nc()
    return _NC


def _permute_cats(c):
    """Build the dma_gather index stream for the partition-major layout.

    Stream index s = col*128 + p (col = global position-per-partition)
    must hold cats[p*C + col]. Wrap (index s at [s%16, s//16]) and
    replicate across the 8 16-partition groups.
    """
    a = np.ascontiguousarray(c.reshape(P, C).T).reshape(N)   # [col, p] flat
    return np.ascontiguousarray(np.tile(a.reshape(N // 16, 16).T, (8, 1)))


def _shard_inputs(inputs, categories, emb_table):
    tab = np.asarray(emb_table, dtype=BF16)
    tab[0, :] = BF16(1.0)       # padding rows (cat==0) multiply by 1.0
    tab = np.ascontiguousarray(tab)
    in_maps = []
    for i in range(N_CORES):
        xs = np.ascontiguousarray(
            np.asarray(inputs[i * B_LOC:(i + 1) * B_LOC], dtype=BF16)
        ).reshape(N, D)
        c = categories[i * B_LOC:(i + 1) * B_LOC].reshape(N).astype(np.int16)
        in_maps.append({"x": xs, "cats16": _permute_cats(c), "table": tab})
    return in_maps


def kernel(inputs, categories, mask_positions=None, emb_table=None, **_):
    """Full (unsharded) inputs in, full output out. mask_positions unused."""
    nc = _get_nc()
    in_maps = _shard_inputs(inputs, categories, emb_table)
    res = run_bass_kernel_spmd(nc, in_maps, list(range(N_CORES)))
    out = np.empty((B, S, D), dtype=np.float32)
    for i in range(N_CORES):
        out[i * B_LOC:(i + 1) * B_LOC] = (
            res.results[i]["y"].astype(np.float32).reshape(B_LOC, S, D)
        )
    return out


# revision 4
# speedup vs baseline: 2.9474x; 2.9474x over previous
"""Trainium2 Bass kernel for nn_CategoryMultiplier.

out[b, s, :] = inputs[b, s, :] * (emb_table[categories[b, s]] if
               categories[b, s] != 0 else 1.0)

Sharding: pure data parallel over batch. 8 cores x 16 batches each.

All device I/O is bf16 (harness gate is rel_err < 2e-2; bf16 rounding of
x, table and y is ~0.4% worst case): per core x flat [8192, 512] bf16,
table [1000, 512] bf16 (row 0 pre-set to ones on host so padding
positions multiply by 1.0), y [8192, 512] bf16 upcast to f32 on host.
This halves HBM/DMA traffic vs f32 (24MB/core vs 48MB/core).

Device layout: positions are partition-major (partition p holds positions
p*64 .. p*64+63) so the input/output DMAs use contiguous descriptors per
partition. Embedding rows are fetched with one InstDMAGatherAnt per chunk;
its fixed dst layout dst[i%128, i//128] is reconciled with the
partition-major layout by permuting the index array on the host (pure
layout prep).
"""

import numpy as np
import ml_dtypes

import concourse.bass as bass
import concourse.bacc as bacc
import concourse.mybir as mybir
import concourse.tile as tile
from concourse.bass_utils import run_bass_kernel_spmd

BF16 = ml_dtypes.bfloat16

# Problem shape (hardcoded per harness contract).
B, S, D = 128, 512, 512
VOCAB = 1000
N_CORES = 8
B_LOC = B // N_CORES            # 16 batches per core
N = B_LOC * S                   # 8192 positions per core
P = 128                         # SBUF partitions
C = N // P                      # 64 positions per partition
T_CH = 8                        # max positions-per-partition per chunk

MBF16 = mybir.dt.bfloat16
I16 = mybir.dt.int16

# Small chunks at the head (one per SWDGE queue) prime the pipeline.
CHUNKS = [4, 4, 4, 4] + [8] * 6
assert sum(CHUNKS) == C

# dma_gather desc-gen runs on the Q7 core pair selected by queue_num
# (dma_gather.cpp: cpu_id/2 == queue_num), so round-robin over all 4
# SWDGE queues lets 4 gathers generate descriptors concurrently.
N_Q = 4


def _build_nc():
    nc = bacc.Bacc("TRN2", target_bir_lowering=False, debug=False,
                   num_swdge_queues=N_Q)

    x = nc.dram_tensor("x", [N, D], MBF16, kind="ExternalInput")
    cats16 = nc.dram_tensor("cats16", [P, N // 16], I16, kind="ExternalInput")
    table = nc.dram_tensor("table", [VOCAB, D], MBF16, kind="ExternalInput")
    y = nc.dram_tensor("y", [N, D], MBF16, kind="ExternalOutput")

    xr = x[:].rearrange("(p c) d -> p (c d)", p=P)     # [128, C*D]
    yr = y[:].rearrange("(p c) d -> p (c d)", p=P)

    # Issue the GPSIMD ucode library load BEFORE the TileContext so the
    # ~14us IRAM load overlaps Tile's own prologue barrier instead of
    # running after it.
    from concourse.library_config import mlp
    nc.gpsimd.load_library(mlp)

    with tile.TileContext(nc) as tc:
        with (
            tc.tile_pool(name="const", bufs=1) as const_pool,
            tc.tile_pool(name="io", bufs=5) as io_pool,
            tc.tile_pool(name="gat", bufs=6) as gat_pool,
        ):
            cats_t = const_pool.tile([P, N // 16], I16)
            nc.scalar.dma_start(out=cats_t[:], in_=cats16[:])

            pos = 0
            for ci, tch in enumerate(CHUNKS):
                lo, hi = pos * D, (pos + tch) * D
                n_idx = tch * P
                g_t = gat_pool.tile([P, T_CH * D], MBF16, tag="g")
                nc.gpsimd.dma_gather(
                    out_ap=g_t[:, :tch * D].rearrange("p (t d) -> p t d", t=tch),
                    in_ap=table[:],
                    idxs_ap=cats_t[:, pos * 8:(pos + tch) * 8],
                    num_idxs=n_idx,
                    num_idxs_reg=n_idx,
                    elem_size=D,
                    queue_num=ci % N_Q,
                )

                x_t = io_pool.tile([P, T_CH * D], MBF16, tag="x")
                nc.sync.dma_start(out=x_t[:, :tch * D], in_=xr[:, lo:hi])

                nc.vector.tensor_mul(out=g_t[:, :tch * D], in0=g_t[:, :tch * D],
                                     in1=x_t[:, :tch * D])
                nc.scalar.dma_start(out=yr[:, lo:hi], in_=g_t[:, :tch * D])
                pos += tch

    nc.compile()
    return nc


_NC = None


def _get_nc():
    global _NC
    if _NC is None:
        _NC = _build_nc()
    return _NC


def _permute_cats(c):
    """Build the dma_gather index stream for the partition-major layout.

    Stream index s = col*128 + p (col = global position-per-partition)
    must hold cats[p*C + col]. Wrap (index s at [s%16, s//16]) and
    replicate across the 8 16-partition groups.
    """
    a = np.ascontiguousarray(c.reshape(P, C).T).reshape(N)   # [col, p] flat
    return np.ascontiguousarray(np.tile(a.reshape(N // 16, 16).T, (8, 1)))


def _shard_inputs(inputs, categories, emb_table):
    tab = np.asarray(emb_table, dtype=BF16)
    tab[0, :] = BF16(1.0)       # padding rows (cat==0) multiply by 1.0
    tab = np.ascontiguousarray(tab)
    in_maps = []
    for i in range(N_CORES):
        xs = np.ascontiguousarray(
            np.asarray(inputs[i * B_LOC:(i + 1) * B_LOC], dtype=BF16)
        ).reshape(N, D)
        c = categories[i * B_LOC:(i + 1) * B_LOC].reshape(N).astype(np.int16)
        in_maps.append({"x": xs, "cats16": _permute_cats(c), "table": tab})
    return in_maps


def kernel(inputs, categories, mask_positions=None, emb_table=None, **_):
    """Full (unsharded) inputs in, full output out. mask_positions unused."""
    nc = _get_nc()
    in_maps = _shard_inputs(inputs, categories, emb_table)
    res = run_bass_kernel_spmd(nc, in_maps, list(range(N_CORES)))
    out = np.empty((B, S, D), dtype=np.float32)
    for i in range(N_CORES):
        out[i * B_LOC:(i + 1) * B_LOC] = (
            res.results[i]["y"].astype(np.float32).reshape(B_LOC, S, D)
        )
    return out


# revision 7
# speedup vs baseline: 3.2455x; 1.1011x over previous
"""Trainium2 Bass kernel for nn_CategoryMultiplier.

out[b, s, :] = inputs[b, s, :] * (emb_table[categories[b, s]] if
               categories[b, s] != 0 else 1.0)

Sharding: pure data parallel over batch. 8 cores x 16 batches each.

All device I/O is bf16 (harness gate is rel_err < 2e-2; bf16 rounding of
x, table and y is ~0.4% worst case): per core x flat [8192, 512] bf16,
table [1000, 512] bf16 (row 0 pre-set to ones on host so padding
positions multiply by 1.0), y [8192, 512] bf16 upcast to f32 on host.
This halves HBM/DMA traffic vs f32 (24MB/core vs 48MB/core).

Device layout: positions are partition-major (partition p holds positions
p*64 .. p*64+63) so the input/output DMAs use contiguous descriptors per
partition. Embedding rows are fetched with one InstDMAGatherAnt per chunk;
its fixed dst layout dst[i%128, i//128] is reconciled with the
partition-major layout by permuting the index array on the host (pure
layout prep).
"""

import numpy as np
import ml_dtypes

import concourse.bass as bass
import concourse.bacc as bacc
import concourse.mybir as mybir
import concourse.tile as tile
from concourse.bass_utils import run_bass_kernel_spmd

BF16 = ml_dtypes.bfloat16

# Problem shape (hardcoded per harness contract).
B, S, D = 128, 512, 512
VOCAB = 1000
N_CORES = 8
B_LOC = B // N_CORES            # 16 batches per core
N = B_LOC * S                   # 8192 positions per core
P = 128                         # SBUF partitions
C = N // P                      # 64 positions per partition
T_CH = 4                        # max positions-per-partition per chunk

MBF16 = mybir.dt.bfloat16
I16 = mybir.dt.int16

# Uniform fine-grained chunks: tight DMA/desc-gen/mul overlap and a
# short (~5us) drain tail.
CHUNKS = [4] * 16
assert sum(CHUNKS) == C

# dma_gather desc-gen runs on the Q7 core pair selected by queue_num
# (dma_gather.cpp: cpu_id/2 == queue_num), so round-robin over all 4
# SWDGE queues lets 4 gathers generate descriptors concurrently.
N_Q = 4


def _build_nc():
    nc = bacc.Bacc("TRN2", target_bir_lowering=False, debug=False,
                   num_swdge_queues=N_Q)

    x = nc.dram_tensor("x", [N, D], MBF16, kind="ExternalInput")
    cats16 = nc.dram_tensor("cats16", [P, N // 16], I16, kind="ExternalInput")
    table = nc.dram_tensor("table", [VOCAB, D], MBF16, kind="ExternalInput")
    y = nc.dram_tensor("y", [N, D], MBF16, kind="ExternalOutput")

    xr = x[:].rearrange("(p c) d -> p (c d)", p=P)     # [128, C*D]
    yr = y[:].rearrange("(p c) d -> p (c d)", p=P)

    # Issue the GPSIMD ucode library load BEFORE the TileContext so the
    # ~14us IRAM load overlaps Tile's own prologue barrier instead of
    # running after it.
    from concourse.library_config import mlp
    nc.gpsimd.load_library(mlp)

    with tile.TileContext(nc) as tc:
        with (
            tc.tile_pool(name="const", bufs=1) as const_pool,
            tc.tile_pool(name="io", bufs=8) as io_pool,
            tc.tile_pool(name="gat", bufs=8) as gat_pool,
        ):
            cats_t = const_pool.tile([P, N // 16], I16)
            nc.scalar.dma_start(out=cats_t[:], in_=cats16[:])

            pos = 0
            for ci, tch in enumerate(CHUNKS):
                lo, hi = pos * D, (pos + tch) * D
                n_idx = tch * P
                g_t = gat_pool.tile([P, T_CH * D], MBF16, tag="g")
                nc.gpsimd.dma_gather(
                    out_ap=g_t[:, :tch * D].rearrange("p (t d) -> p t d", t=tch),
                    in_ap=table[:],
                    idxs_ap=cats_t[:, pos * 8:(pos + tch) * 8],
                    num_idxs=n_idx,
                    num_idxs_reg=n_idx,
                    elem_size=D,
                    queue_num=ci % N_Q,
                )

                x_t = io_pool.tile([P, T_CH * D], MBF16, tag="x")
                nc.sync.dma_start(out=x_t[:, :tch * D], in_=xr[:, lo:hi])

                nc.vector.tensor_mul(out=g_t[:, :tch * D], in0=g_t[:, :tch * D],
                                     in1=x_t[:, :tch * D])
                nc.scalar.dma_start(out=yr[:, lo:hi], in_=g_t[:, :tch * D])
                pos += tch

    nc.compile()
    return nc


_NC = None


def _get_nc():
    global _NC
    if _NC is None:
        _NC = _build_nc()
    return _NC


def _permute_cats(c):
    """Build the dma_gather index stream for the partition-major layout.

    Stream index s = col*128 + p (col = global position-per-partition)
    must hold cats[p*C + col]. Wrap (index s at [s%16, s//16]) and
    replicate across the 8 16-partition groups.
    """
    a = np.ascontiguousarray(c.reshape(P, C).T).reshape(N)   # [col, p] flat
    return np.ascontiguousarray(np.tile(a.reshape(N // 16, 16).T, (8, 1)))


def _shard_inputs(inputs, categories, emb_table):
    tab = np.asarray(emb_table, dtype=BF16)
    tab[0, :] = BF16(1.0)       # padding rows (cat==0) multiply by 1.0
    tab = np.ascontiguousarray(tab)
    in_maps = []
    for i in range(N_CORES):
        xs = np.ascontiguousarray(
            np.asarray(inputs[i * B_LOC:(i + 1) * B_LOC], dtype=BF16)
        ).reshape(N, D)
        c = categories[i * B_LOC:(i + 1) * B_LOC].reshape(N).astype(np.int16)
        in_maps.append({"x": xs, "cats16": _permute_cats(c), "table": tab})
    return in_maps


def kernel(inputs, categories, mask_positions=None, emb_table=None, **_):
    """Full (unsharded) inputs in, full output out. mask_positions unused."""
    nc = _get_nc()
    in_maps = _shard_inputs(inputs, categories, emb_table)
    res = run_bass_kernel_spmd(nc, in_maps, list(range(N_CORES)))
    out = np.empty((B, S, D), dtype=np.float32)
    for i in range(N_CORES):
        out[i * B_LOC:(i + 1) * B_LOC] = (
            res.results[i]["y"].astype(np.float32).reshape(B_LOC, S, D)
        )
    return out


# revision 9
# speedup vs baseline: 3.3049x; 1.0183x over previous
"""Trainium2 Bass kernel for nn_CategoryMultiplier.

out[b, s, :] = inputs[b, s, :] * (emb_table[categories[b, s]] if
               categories[b, s] != 0 else 1.0)

Sharding: pure data parallel over batch. 8 cores x 16 batches each.

All device I/O is bf16 (harness gate is rel_err < 2e-2; bf16 rounding of
x, table and y is ~0.4% worst case): per core x flat [8192, 512] bf16,
table [1000, 512] bf16 (row 0 pre-set to ones on host so padding
positions multiply by 1.0), y [8192, 512] bf16 upcast to f32 on host.
This halves HBM/DMA traffic vs f32 (24MB/core vs 48MB/core).

Device layout: positions are partition-major (partition p holds positions
p*64 .. p*64+63) so the input/output DMAs use contiguous descriptors per
partition. Embedding rows are fetched with one InstDMAGatherAnt per chunk;
its fixed dst layout dst[i%128, i//128] is reconciled with the
partition-major layout by permuting the index array on the host (pure
layout prep).
"""

import numpy as np
import ml_dtypes

import concourse.bass as bass
import concourse.bacc as bacc
import concourse.mybir as mybir
import concourse.tile as tile
from concourse.bass_utils import run_bass_kernel_spmd

BF16 = ml_dtypes.bfloat16

# Problem shape (hardcoded per harness contract).
B, S, D = 128, 512, 512
VOCAB = 1000
N_CORES = 8
B_LOC = B // N_CORES            # 16 batches per core
N = B_LOC * S                   # 8192 positions per core
P = 128                         # SBUF partitions
C = N // P                      # 64 positions per partition
T_CH = 4                        # max positions-per-partition per chunk

MBF16 = mybir.dt.bfloat16
I16 = mybir.dt.int16

# Uniform fine-grained chunks: tight DMA/desc-gen/mul overlap; small
# final chunks shorten the drain tail.
CHUNKS = [4] * 15 + [2, 2]
assert sum(CHUNKS) == C

# dma_gather desc-gen runs on the Q7 core pair selected by queue_num
# (dma_gather.cpp: cpu_id/2 == queue_num), so round-robin over all 4
# SWDGE queues lets 4 gathers generate descriptors concurrently.
N_Q = 4


def _build_nc():
    nc = bacc.Bacc("TRN2", target_bir_lowering=False, debug=False,
                   num_swdge_queues=N_Q)

    x = nc.dram_tensor("x", [N, D], MBF16, kind="ExternalInput")
    cats16 = nc.dram_tensor("cats16", [P, N // 16], I16, kind="ExternalInput")
    table = nc.dram_tensor("table", [VOCAB, D], MBF16, kind="ExternalInput")
    y = nc.dram_tensor("y", [N, D], MBF16, kind="ExternalOutput")

    xr = x[:].rearrange("(p c) d -> p (c d)", p=P)     # [128, C*D]
    yr = y[:].rearrange("(p c) d -> p (c d)", p=P)

    # Issue the GPSIMD ucode library load BEFORE the TileContext so the
    # ~14us IRAM load overlaps Tile's own prologue barrier instead of
    # running after it.
    from concourse.library_config import mlp
    nc.gpsimd.load_library(mlp)

    with tile.TileContext(nc) as tc:
        with (
            tc.tile_pool(name="const", bufs=1) as const_pool,
            # io deep enough to stream ALL of x during the ~15us GPSIMD
            # library-load window (gathers can't start until it finishes).
            tc.tile_pool(name="io", bufs=17) as io_pool,
            tc.tile_pool(name="gat", bufs=8) as gat_pool,
        ):
            cats_t = const_pool.tile([P, N // 16], I16)
            nc.scalar.dma_start(out=cats_t[:], in_=cats16[:])

            pos = 0
            for ci, tch in enumerate(CHUNKS):
                lo, hi = pos * D, (pos + tch) * D
                n_idx = tch * P
                g_t = gat_pool.tile([P, T_CH * D], MBF16, tag="g")
                nc.gpsimd.dma_gather(
                    out_ap=g_t[:, :tch * D].rearrange("p (t d) -> p t d", t=tch),
                    in_ap=table[:],
                    idxs_ap=cats_t[:, pos * 8:(pos + tch) * 8],
                    num_idxs=n_idx,
                    num_idxs_reg=n_idx,
                    elem_size=D,
                    queue_num=ci % N_Q,
                )

                x_t = io_pool.tile([P, T_CH * D], MBF16, tag="x")
                nc.sync.dma_start(out=x_t[:, :tch * D], in_=xr[:, lo:hi])

                nc.vector.tensor_mul(out=g_t[:, :tch * D], in0=g_t[:, :tch * D],
                                     in1=x_t[:, :tch * D])
                nc.scalar.dma_start(out=yr[:, lo:hi], in_=g_t[:, :tch * D])
                pos += tch

    nc.compile()
    return nc


_NC = None


def _get_nc():
    global _NC
    if _NC is None:
        _NC = _build_nc()
    return _NC


def _permute_cats(c):
    """Build the dma_gather index stream for the partition-major layout.

    Stream index s = col*128 + p (col = global position-per-partition)
    must hold cats[p*C + col]. Wrap (index s at [s%16, s//16]) and
    replicate across the 8 16-partition groups.
    """
    a = np.ascontiguousarray(c.reshape(P, C).T).reshape(N)   # [col, p] flat
    return np.ascontiguousarray(np.tile(a.reshape(N // 16, 16).T, (8, 1)))


def _shard_inputs(inputs, categories, emb_table):
    tab = np.asarray(emb_table, dtype=BF16)
    tab[0, :] = BF16(1.0)       # padding rows (cat==0) multiply by 1.0
    tab = np.ascontiguousarray(tab)
    in_maps = []
    for i in range(N_CORES):
        xs = np.ascontiguousarray(
            np.asarray(inputs[i * B_LOC:(i + 1) * B_LOC], dtype=BF16)
        ).reshape(N, D)
        c = categories[i * B_LOC:(i + 1) * B_LOC].reshape(N).astype(np.int16)
        in_maps.append({"x": xs, "cats16": _permute_cats(c), "table": tab})
    return in_maps


def kernel(inputs, categories, mask_positions=None, emb_table=None, **_):
    """Full (unsharded) inputs in, full output out. mask_positions unused."""
    nc = _get_nc()
    in_maps = _shard_inputs(inputs, categories, emb_table)
    res = run_bass_kernel_spmd(nc, in_maps, list(range(N_CORES)))
    out = np.empty((B, S, D), dtype=np.float32)
    for i in range(N_CORES):
        out[i * B_LOC:(i + 1) * B_LOC] = (
            res.results[i]["y"].astype(np.float32).reshape(B_LOC, S, D)
        )
    return out
